# revision 22
# baseline (speedup 1.0000x reference)
"""Fused Mllama-style text self-attention on one TRN2 chip (8 NeuronCores).

Sharding: tensor-parallel over heads (4 q heads / 1 kv head per core) for the
QKV projections + RoPE + attention; per-(head, batch) AllToAlls reshard the
attention outputs to token-parallel (tokens interleaved across cores by
batch), so each core computes a 512-token slice of the final output
projection against the full wo.

Precision strategy: the Q/K projections run in fp8e4 with DoubleRow matmuls
(scores feed a softmax whose inputs are tiny, so score error washes out);
V, attention, and the output projection stay bf16.

All large DRAM operands are pre-tiled on the host so that every SBUF load is
a single contiguous [128, N] DMA.

kernel(**inputs) takes the FULL (unsharded) inputs and returns the FULL output.
"""

import math

import numpy as np
import ml_dtypes

import concourse.bacc as bacc
import concourse.bass as bass
import concourse.mybir as mybir
import concourse.tile as tile
from concourse.bass_utils import run_bass_kernel_spmd

F32 = mybir.dt.float32
F32R = mybir.dt.float32r
BF16 = mybir.dt.bfloat16
FP8 = mybir.dt.float8e4
AF = mybir.ActivationFunctionType
ALU = mybir.AluOpType
DR = mybir.MatmulPerfMode.DoubleRow

NH, NKV, HD = 32, 8, 128
NEG = -1.0e9
N_CORES = 8

# fp8 scaling for the q/k projections: host multiplies hs and wq/wk by 32;
# the rope cos/sin tables carry the 1/1024 correction.
S_HS = 32.0
S_W = 32.0
S_QKV = S_HS * S_W


def build(T, S, H, compute_dtype="fp8", causal=True, n_cores=N_CORES):
    """Build the SPMD Bass program (same program for all cores)."""
    B = T // S
    TC = T // n_cores          # tokens per core in the output projection
    TB = TC // B               # tokens per (core, batch)
    QHC = NH // n_cores        # local q heads (4)
    D = QHC * HD               # local q width (512)
    HT = H // 128              # contraction tiles over hidden
    QB = min(512, TC)          # attention query block width
    NQB = S // QB              # query blocks per batch
    KB = QB // 128             # 128-k-tiles per query block
    NKT = S // 128             # k tiles per batch
    NMB = H // 512             # output-projection column blocks
    NT = TC // 128             # output-projection row tiles
    fp8 = compute_dtype == "fp8"
    CD = BF16                  # on-chip attention / v / out-proj dtype
    QKD = FP8 if fp8 else BF16  # q/k projection operand dtype
    ISQ = 1.0 / math.sqrt(HD)
    TI = 512                   # tokens per QKV iteration
    NIT = T // TI
    nh2 = HT // 2
    NGQ = NH // 4              # phase-3 head quads

    nc = bacc.Bacc("TRN2", target_bir_lowering=False, debug=False,
                   enable_asserts=True, num_devices=n_cores)

    # pre-tiled [block, 128, cols] operands (host prepares the layouts)
    hsTt = nc.declare_dram_parameter("hsTt", [NIT * 2, 128, nh2 * TI], CD,
                                     isOutput=False)
    if fp8:
        hsT8t = nc.declare_dram_parameter("hsT8t", [NIT * 2, 128, nh2 * TI],
                                          FP8, isOutput=False)
    wqTt = nc.declare_dram_parameter("wqTt", [128, HT * D], QKD, isOutput=False)
    wkTt = nc.declare_dram_parameter("wkTt", [128, HT * HD], QKD, isOutput=False)
    wvTt = nc.declare_dram_parameter("wvTt", [128, HT * HD], CD, isOutput=False)
    woTt = nc.declare_dram_parameter("woTt", [NMB // 2 * NGQ, 128, 4 * 1024],
                                     CD, isOutput=False)
    cosT = nc.declare_dram_parameter("cosT", [HD, S], F32, isOutput=False)
    sgnT = nc.declare_dram_parameter("sgnT", [HD, S], F32, isOutput=False)
    idn = nc.declare_dram_parameter("idn", [128, 128], CD, isOutput=False)
    if causal:
        dmask = nc.declare_dram_parameter("dmask", [128, KB * QB], CD, isOutput=False)
    else:
        maskT = nc.declare_dram_parameter("maskT", [S, S], F32, isOutput=False)
    out_c = nc.declare_dram_parameter("out", [TC, H], F32, isOutput=True)

    with tile.TileContext(nc) as tc:
        with tc.tile_pool(name="persist", bufs=1) as per, \
             tc.tile_pool(name="dram", bufs=1, space="DRAM") as dram:
            # persistent SBUF tensors
            qt = per.tile([128, QHC * T], CD)      # rope'd Q, head-major [d, t]
            kt = per.tile([128, T], CD)            # rope'd K [d, t]
            vt = per.tile([128, T], CD)            # V tiles [t(128), d] at col k*128
            cs = per.tile([128, S], F32)
            sg = per.tile([128, S], F32)
            ident = per.tile([128, 128], CD)
            ones_c = per.tile([128, 1], CD)        # den-matmul stationary
            ones_f32 = per.tile([1, 128], F32)
            ones_fr = per.tile([1, 128], F32R)     # bcast-matmul stationary
            nc.gpsimd.memset(ones_c[:], 1.0)
            nc.gpsimd.memset(ones_f32[:], 1.0)
            nc.scalar.activation(ones_fr[:], ones_f32[:], AF.Copy)
            # pre-warm the exp table set during the initial DMA wait so the
            # ~2.7us ACT_TABLE_LOAD is off the attention critical path
            warm = per.tile([1, 1], F32)
            nc.scalar.activation(warm[:], ones_f32[:, 0:1], AF.Exp)
            if causal:
                dm = per.tile([128, KB * QB], CD)

            # per-(head, batch) A2A bounce buffers; tokens interleaved by
            # batch so each batch ships in its own half-size collective
            a2a_in = [[dram.tile([n_cores, 128, TB], CD,
                                 name=f"a2a_in{i}_{b}") for b in range(B)]
                      for i in range(QHC)]
            a2a_out = [[dram.tile([n_cores, 128, TB], CD,
                                  name=f"a2a_out{i}_{b}") for b in range(B)]
                       for i in range(QHC)]

            # ---------------- Phase 1: QKV projections + RoPE ----------------
            with tc.tile_pool(name="wq", bufs=1) as wqp, \
                 tc.tile_pool(name="hst", bufs=3) as hstp, \
                 tc.tile_pool(name="qkps", bufs=2, space="PSUM") as qkps, \
                 tc.tile_pool(name="vps", bufs=2, space="PSUM") as vps, \
                 tc.tile_pool(name="epi", bufs=3) as epi:
                wq_sb = wqp.tile([128, HT * D], QKD)
                wk_sb = wqp.tile([128, HT * HD], QKD)
                wv_sb = wqp.tile([128, HT * HD], CD)
                # split the big weight loads so the first matmuls only wait
                # on their own quarter of the table
                wqc = HT * D // 4
                for q4 in range(4):
                    nc.sync.dma_start(wq_sb[:, q4 * wqc:(q4 + 1) * wqc],
                                      wqTt[:, q4 * wqc:(q4 + 1) * wqc])
                # off-critical-path loads ride the scalar HWDGE ring so the
                # sync ring streams only the wq quarters + hs tiles the first
                # matmuls wait on; these are first needed ~20us in
                nc.scalar.dma_start(wk_sb[:], wkTt[:])
                nc.scalar.dma_start(wv_sb[:], wvTt[:])
                nc.scalar.dma_start(cs[:], cosT[:])
                nc.scalar.dma_start(sg[:], sgnT[:])
                nc.scalar.dma_start(ident[:], idn[:])
                if causal:
                    nc.scalar.dma_start(dm[:], dmask[:])
                wq_r = wq_sb[:].rearrange("p (ht d) -> p ht d", ht=HT)
                wk_r = wk_sb[:].rearrange("p (ht d) -> p ht d", ht=HT)

                def rope(pp, dst_ap, sc):
                    # dst = ab*cos + rotate_half(ab)*sin with ab = pp (psum).
                    # The half-rotation crosses partitions, which compute
                    # engines cannot do SBUF->SBUF, so shift via on-chip DMA.
                    ab = epi.tile([128, TI], F32, tag="ab", name="ab")
                    nc.scalar.activation(ab[:], pp[:], AF.Copy)
                    sh = epi.tile([128, TI], F32, tag="sh", name="sh")
                    # scalar-engine HWDGE ring keeps these small shifts off
                    # the sync ring that streams the hs tiles
                    nc.scalar.dma_start(sh[0:64, :], ab[64:128, :])
                    nc.scalar.dma_start(sh[64:128, :], ab[0:64, :])
                    x1 = epi.tile([128, TI], F32, tag="x1", name="x1")
                    nc.vector.tensor_mul(x1[:], ab[:], cs[:, sc:sc + TI])
                    nc.vector.tensor_mul(sh[:], sh[:], sg[:, sc:sc + TI])
                    nc.vector.tensor_add(dst_ap, x1[:], sh[:])

                for it in range(NIT):
                    t0 = it * TI
                    sc = t0 % S  # column into cos/sgn tables
                    hsp8_halves = []
                    hsp_halves = []
                    # fp8 tiles load first: the q/k DoubleRow matmuls lead
                    # each iteration, the bf16 v matmuls trail it
                    if fp8:
                        for half in range(2):
                            hsp8 = hstp.tile([128, nh2 * TI], FP8, tag="hsp8",
                                             name=f"hsp8_{it}_{half}")
                            nc.sync.dma_start(hsp8[:], hsT8t[2 * it + half])
                            hsp8_halves.append(hsp8)
                    for half in range(2):
                        hsp = hstp.tile([128, nh2 * TI], CD, tag="hsp",
                                        name=f"hsp_{it}_{half}")
                        nc.sync.dma_start(hsp[:], hsTt[2 * it + half])
                        hsp_halves.append(hsp)

                    def hs_t(ht):
                        h = hsp_halves[ht // nh2]
                        j = ht % nh2
                        return h[:, j * TI:(j + 1) * TI]

                    def hs8_pair(j):
                        # [128, 2, TI] for DoubleRow (both ht in same half)
                        h = hsp8_halves[(2 * j) // nh2]
                        jj = (2 * j) % nh2
                        return h[:].rearrange("p (ht t) -> p ht t",
                                              ht=nh2)[:, jj:jj + 2, :]

                    for g in range(QHC + 1):  # 4 q heads, then k
                        pp = qkps.tile([128, TI], F32, tag="pp",
                                       name=f"pp_{it}_{g}")
                        if fp8:
                            for j in range(HT // 2):
                                if g < QHC:
                                    w_ap = wq_r[:, 2 * j:2 * j + 2,
                                                g * 128:(g + 1) * 128]
                                else:
                                    w_ap = wk_r[:, 2 * j:2 * j + 2, :]
                                nc.tensor.matmul(pp[:], w_ap, hs8_pair(j),
                                                 start=(j == 0),
                                                 stop=(j == HT // 2 - 1),
                                                 perf_mode=DR)
                        else:
                            for ht in range(HT):
                                if g < QHC:
                                    w_ap = wq_sb[:, ht * D + g * 128:
                                                 ht * D + (g + 1) * 128]
                                else:
                                    w_ap = wk_sb[:, ht * HD:(ht + 1) * HD]
                                nc.tensor.matmul(pp[:], w_ap, hs_t(ht),
                                                 start=(ht == 0),
                                                 stop=(ht == HT - 1))
                        if g < QHC:
                            rope(pp, qt[:, g * T + t0: g * T + t0 + TI], sc)
                        else:
                            rope(pp, kt[:, t0:t0 + TI], sc)
                    # v (always bf16): [d, t] accumulation, then transpose
                    pp = qkps.tile([128, TI], F32, tag="pp", name=f"ppv_{it}")
                    for ht in range(HT):
                        w_ap = wv_sb[:, ht * HD:(ht + 1) * HD]
                        nc.tensor.matmul(pp[:], w_ap, hs_t(ht),
                                         start=(ht == 0), stop=(ht == HT - 1))
                    vdt = epi.tile([128, TI], CD, tag="vdt", name="vdt")
                    nc.scalar.activation(vdt[:], pp[:], AF.Copy)
                    vtp = vps.tile([128, TI], CD, tag="vtp", name=f"vtp_{it}")
                    for tsub in range(TI // 128):
                        nc.tensor.transpose(
                            vtp[:, tsub * 128:(tsub + 1) * 128],
                            vdt[:, tsub * 128:(tsub + 1) * 128],
                            ident[:])
                    nc.vector.tensor_copy(vt[:, t0:t0 + TI], vtp[:])

            # no barrier: attention's leading QK/exp work may overlap the
            # projection tail (PSUM/SBUF pool releases gate the rest)
            # ---------------- Phase 2: attention ----------------
            with tc.tile_pool(name="stps", bufs=2, space="PSUM") as stps, \
                 tc.tile_pool(name="otps", bufs=2, space="PSUM") as otps, \
                 tc.tile_pool(name="dbps", bufs=1, space="PSUM") as dbps, \
                 tc.tile_pool(name="att", bufs=8) as att, \
                 tc.tile_pool(name="attm", bufs=3) as attm, \
                 tc.tile_pool(name="accp", bufs=2) as accp:
                for hl in range(QHC):
                    for b in range(B):
                        for qb in range(NQB):
                            q0 = b * S + qb * QB          # global q col
                            n_k = (qb + 1) * KB if causal else NKT
                            otp = otps.tile([128, QB], F32, tag="ot",
                                            name=f"ot_{hl}_{b}_{qb}")
                            acc = accp.tile([128, 2 * QB], CD, tag="acc",
                                            name=f"acc_{hl}_{b}_{qb}")
                            for kp in range(n_k // 2):
                                stp = stps.tile([128, 2 * QB], F32, tag="st",
                                                name=f"st_{hl}_{b}_{qb}_{kp}")
                                for half in range(2):
                                    kti = 2 * kp + half
                                    kg = b * NKT + kti
                                    nc.tensor.matmul(
                                        stp[:, half * QB:(half + 1) * QB],
                                        kt[:, kg * 128:(kg + 1) * 128],
                                        qt[:, hl * T + q0: hl * T + q0 + QB],
                                        start=True, stop=True)
                                pt = att.tile([128, 2 * QB], CD, tag="pt",
                                              name=f"pt_{hl}_{b}_{qb}_{kp}")
                                d0 = 2 * kp - qb * KB  # diag pattern index
                                if causal and 2 * kp + 1 >= qb * KB:
                                    nc.scalar.activation(pt[:], stp[:], AF.Exp,
                                                         scale=ISQ)
                                    nc.vector.tensor_mul(
                                        pt[:], pt[:],
                                        dm[:, d0 * QB:(d0 + 2) * QB])
                                elif not causal:
                                    mt = attm.tile([128, 2 * QB], F32, tag="mt",
                                                   name="mt")
                                    for half in range(2):
                                        kti = 2 * kp + half
                                        nc.sync.dma_start(
                                            mt[:, half * QB:(half + 1) * QB],
                                            maskT[kti * 128:(kti + 1) * 128,
                                                  qb * QB:(qb + 1) * QB])
                                    tmp = att.tile([128, 2 * QB], F32, tag="tmp",
                                                   name="tmp")
                                    nc.vector.tensor_add(tmp[:], stp[:], mt[:])
                                    nc.scalar.activation(pt[:], tmp[:], AF.Exp,
                                                         scale=ISQ)
                                else:
                                    nc.scalar.activation(pt[:], stp[:], AF.Exp,
                                                         scale=ISQ)
                                for half in range(2):
                                    kti = 2 * kp + half
                                    kg = b * NKT + kti
                                    nc.tensor.matmul(
                                        otp[:], vt[:, kg * 128:(kg + 1) * 128],
                                        pt[:, half * QB:(half + 1) * QB],
                                        start=(kti == 0), stop=(kti == n_k - 1))
                                if kp == 0:
                                    nc.vector.tensor_copy(acc[:], pt[:])
                                else:
                                    nc.vector.tensor_add(acc[:], acc[:], pt[:])
                            # denominator: partition-reduce the acc chain on
                            # PE, bcast, fast reciprocal, normalize
                            db = dbps.tile([1, QB], F32, tag="db",
                                           name=f"db_{hl}_{b}_{qb}")
                            nc.tensor.matmul(db[0:1, :], ones_c[:],
                                             acc[:, 0:QB],
                                             start=True, stop=False)
                            nc.tensor.matmul(db[0:1, :], ones_c[:],
                                             acc[:, QB:2 * QB],
                                             start=False, stop=True)
                            dsb = att.tile([1, QB], F32R, tag="dsb", name="dsb")
                            nc.vector.tensor_copy(dsb[:], db[0:1, :])
                            dbc = dbps.tile([128, QB], F32, tag="dbc",
                                            name=f"dbc_{hl}_{b}_{qb}")
                            nc.tensor.matmul(dbc[:], ones_fr[:], dsb[:],
                                             start=True, stop=True)
                            rec = att.tile([128, QB], F32, tag="rec", name="rec")
                            nc.vector.reciprocal_approx_fast(rec[:], dbc[:])
                            ot_sb = att.tile([128, QB], CD, tag="otsb",
                                             name="otsb")
                            nc.vector.tensor_mul(ot_sb[:], otp[:], rec[:])
                            # scatter into this (head, batch) A2A buffer
                            tb0 = qb * QB
                            nj = max(1, QB // TB)
                            j0 = tb0 // TB
                            c0 = tb0 % TB
                            if nj == 1:
                                nc.sync.dma_start(
                                    a2a_in[hl][b][j0, :, c0:c0 + QB], ot_sb[:])
                            else:
                                for j in range(nj):
                                    nc.sync.dma_start(
                                        a2a_in[hl][b][j0 + j, :, :],
                                        ot_sb[:, j * TB:(j + 1) * TB])
                        nc.gpsimd.collective_compute(
                            "AllToAll", ALU.bypass,
                            replica_groups=[list(range(n_cores))],
                            ins=[a2a_in[hl][b][:]],
                            outs=[a2a_out[hl][b][:]])

            # no barrier: phase 3's gathers and weight loads overlap the
            # attention tail; its matmuls wait on PSUM release + data deps
            # ---------------- Phase 3: output projection ----------------
            with tc.tile_pool(name="otl", bufs=1) as otlp, \
                 tc.tile_pool(name="wot", bufs=6) as wotp, \
                 tc.tile_pool(name="ops", bufs=1, space="PSUM") as ops, \
                 tc.tile_pool(name="oout", bufs=6) as ooutp:
                ot_loc = otlp.tile([128, NH * TC], CD)
                d_order = [i * QHC + hl for hl in range(QHC) for i in range(n_cores)]
                for g in d_order:
                    for b in range(B):
                        nc.scalar.dma_start(
                            ot_loc[:, g * TC + b * TB:g * TC + (b + 1) * TB],
                            a2a_out[g % QHC][b][g // QHC, :, :])
                for mp in range(NMB // 2):
                    pos = [ops.tile([128, 512], F32, tag=f"po{sub}{tt}",
                                    name=f"po_{mp}_{sub}_{tt}")
                           for sub in range(2) for tt in range(NT)]
                    for gq in range(NGQ):
                        # pre-tiled quad of four heads' wo rows
                        wot = wotp.tile([128, 4 * 1024], CD, tag="wot",
                                        name=f"wot_{mp}_{gq}")
                        nc.sync.dma_start(wot[:], woTt[mp * NGQ + gq])
                        for gi in range(4):
                            g = d_order[gq * 4 + gi]
                            first = gq == 0 and gi == 0
                            last = gq == NGQ - 1 and gi == 3
                            for sub in range(2):
                                for tt in range(NT):
                                    nc.tensor.matmul(
                                        pos[sub * NT + tt],
                                        ot_loc[:, g * TC + tt * 128:
                                               g * TC + (tt + 1) * 128],
                                        wot[:, gi * 1024 + sub * 512:
                                            gi * 1024 + (sub + 1) * 512],
                                        start=first, stop=last)
                    for sub in range(2):
                        for tt in range(NT):
                            ob = ooutp.tile([128, 512], F32, tag="ob", name="ob")
                            nc.scalar.activation(ob[:], pos[sub * NT + tt],
                                                 AF.Copy)
                            nc.scalar.dma_start(
                                out_c[tt * 128:(tt + 1) * 128,
                                      (2 * mp + sub) * 512:(2 * mp + sub + 1) * 512],
                                ob[:])

    nc.compile()
    return nc


def _np16(x):
    return np.asarray(x, dtype=ml_dtypes.bfloat16)


def _np8(x):
    return np.asarray(np.clip(np.asarray(x, np.float32), -224.0, 224.0),
                      dtype=ml_dtypes.float8_e4m3)


def _tile_hs(hsT, NIT, TI, nh2):
    """[H, T] -> [NIT*2, 128, nh2*TI] matching the SBUF hsp layout."""
    H, T = hsT.shape
    x = hsT.reshape(2, nh2, 128, NIT, TI)
    x = np.transpose(x, (3, 0, 2, 1, 4))  # [NIT, half, p, ht, t]
    return np.ascontiguousarray(x.reshape(NIT * 2, 128, nh2 * TI))


def _tile_w(wT, HT, D):
    """[H, D] -> [128, HT*D] matching the SBUF weight layout."""
    return np.ascontiguousarray(
        wT.reshape(HT, 128, D).transpose(1, 0, 2).reshape(128, HT * D))


def _tile_wo(woT, d_order, NMB, NGQ):
    """[NH*HD, H] -> [NMB/2*NGQ, 128, 4096] per-(mp, quad) blocks."""
    blocks = np.empty((NMB // 2 * NGQ, 128, 4096), dtype=woT.dtype)
    for mp in range(NMB // 2):
        for gq in range(NGQ):
            for gi in range(4):
                g = d_order[gq * 4 + gi]
                blocks[mp * NGQ + gq, :, gi * 1024:(gi + 1) * 1024] = \
                    woT[g * 128:(g + 1) * 128, mp * 1024:(mp + 1) * 1024]
    return blocks


def prep_inputs(hidden_states, attention_mask, cos, sin, wq, wk, wv, wo,
                compute_dtype="fp8", n_cores=N_CORES):
    """Host-side sharding + pre-tiling. Returns (in_maps, causal, dims)."""
    B, S, H = hidden_states.shape
    T = B * S
    D = NH * HD // n_cores
    KD = NKV * HD // n_cores
    HT = H // 128
    TI = 512
    NIT = T // TI
    nh2 = HT // 2
    NMB = H // 512
    NGQ = NH // 4
    QHC = NH // n_cores
    fp8 = compute_dtype == "fp8"

    hs2 = np.asarray(hidden_states, np.float32).reshape(T, H)
    hsTt = _tile_hs(_np16(hs2.T), NIT, TI, nh2)
    d_order = [i * QHC + hl for hl in range(QHC) for i in range(n_cores)]
    woTt = _tile_wo(_np16(np.asarray(wo, np.float32).T), d_order, NMB, NGQ)
    rope_scale = (1.0 / S_QKV) if fp8 else 1.0
    cosT = np.ascontiguousarray(np.asarray(cos, np.float32)[0].T * rope_scale)
    sinT = np.ascontiguousarray(np.asarray(sin, np.float32)[0].T) * rope_scale
    sgnT = np.concatenate([-sinT[0:HD // 2], sinT[HD // 2:]], axis=0)
    sgnT = np.ascontiguousarray(sgnT)

    m = np.asarray(attention_mask, np.float32)[0, 0]
    expected = np.where(np.tril(np.ones((S, S), bool)), 0.0, NEG).astype(np.float32)
    causal = bool(np.array_equal(m, expected))

    TC = T // n_cores
    QB = min(512, TC)
    KB = QB // 128
    qkd = (lambda x: _np8(x * S_W)) if fp8 else (lambda x: _np16(x))
    in_maps = []
    for c in range(n_cores):
        im = {
            "hsTt": hsTt,
            "wqTt": _tile_w(qkd(np.asarray(wq, np.float32)[c * D:(c + 1) * D].T),
                            HT, D),
            "wkTt": _tile_w(qkd(np.asarray(wk, np.float32)[c * KD:(c + 1) * KD].T),
                            HT, HD),
            "wvTt": _tile_w(_np16(np.asarray(wv, np.float32)[c * KD:(c + 1) * KD].T),
                            HT, HD),
            "woTt": woTt,
            "cosT": cosT,
            "sgnT": sgnT,
            "idn": _np16(np.eye(128, dtype=np.float32)),
        }
        if fp8:
            im["hsT8t"] = _tile_hs(_np8(hs2.T * S_HS), NIT, TI, nh2)
        if causal:
            pk = np.arange(128)[:, None]
            pq = np.arange(QB)[None, :]
            dmask = np.concatenate(
                [np.where(pk + j * 128 <= pq, 1.0, 0.0) for j in range(KB)],
                axis=1).astype(np.float32)
            im["dmask"] = _np16(dmask)
        else:
            im["maskT"] = np.ascontiguousarray(m.T)
        in_maps.append(im)
    return in_maps, causal, (T, S, H)


_BUILD_CACHE = {}


def kernel(hidden_states, attention_mask, cos, sin, wq, wk, wv, wo,
           compute_dtype="fp8", trace=False):
    B, S, H = hidden_states.shape
    T = B * S
    in_maps, causal, dims = prep_inputs(
        hidden_states, attention_mask, cos, sin, wq, wk, wv, wo,
        compute_dtype=compute_dtype)
    key = (T, S, H, compute_dtype, causal)
    if key not in _BUILD_CACHE:
        _BUILD_CACHE[key] = build(T, S, H, compute_dtype=compute_dtype,
                                  causal=causal)
    nc = _BUILD_CACHE[key]
    res = run_bass_kernel_spmd(nc, in_maps, core_ids=list(range(N_CORES)),
                               trace=trace)
    TC = T // N_CORES
    TB = TC // B
    out = np.empty((T, H), np.float32)
    for c in range(N_CORES):
        oc = res.results[c]["out"]
        for b in range(B):
            out[b * S + c * TB: b * S + (c + 1) * TB] = \
                oc[b * TB:(b + 1) * TB]
    if trace:
        kernel.last_exec_time_ns = res.exec_time_ns
        kernel.last_results = res
    return out.reshape(B, S, H)


# revision 23
# speedup vs baseline: 1.0011x; 1.0011x over previous
"""Fused Mllama-style text self-attention on one TRN2 chip (8 NeuronCores).

Sharding: tensor-parallel over heads (4 q heads / 1 kv head per core) for the
QKV projections + RoPE + attention; per-(head, batch) AllToAlls reshard the
attention outputs to token-parallel (tokens interleaved across cores by
batch), so each core computes a 512-token slice of the final output
projection against the full wo.

Precision strategy: the Q/K projections run in fp8e4 with DoubleRow matmuls
(scores feed a softmax whose inputs are tiny, so score error washes out);
V, attention, and the output projection stay bf16.

All large DRAM operands are pre-tiled on the host so that every SBUF load is
a single contiguous [128, N] DMA.

kernel(**inputs) takes the FULL (unsharded) inputs and returns the FULL output.
"""

import math

import numpy as np
import ml_dtypes

import concourse.bacc as bacc
import concourse.bass as bass
import concourse.mybir as mybir
import concourse.tile as tile
from concourse.bass_utils import run_bass_kernel_spmd

F32 = mybir.dt.float32
F32R = mybir.dt.float32r
BF16 = mybir.dt.bfloat16
FP8 = mybir.dt.float8e4
AF = mybir.ActivationFunctionType
ALU = mybir.AluOpType
DR = mybir.MatmulPerfMode.DoubleRow

NH, NKV, HD = 32, 8, 128
NEG = -1.0e9
N_CORES = 8

# fp8 scaling for the q/k projections: host multiplies hs and wq/wk by 32;
# the rope cos/sin tables carry the 1/1024 correction.
S_HS = 32.0
S_W = 32.0
S_QKV = S_HS * S_W


def build(T, S, H, compute_dtype="fp8", causal=True, n_cores=N_CORES):
    """Build the SPMD Bass program (same program for all cores)."""
    B = T // S
    TC = T // n_cores          # tokens per core in the output projection
    TB = TC // B               # tokens per (core, batch)
    QHC = NH // n_cores        # local q heads (4)
    D = QHC * HD               # local q width (512)
    HT = H // 128              # contraction tiles over hidden
    QB = min(512, TC)          # attention query block width
    NQB = S // QB              # query blocks per batch
    KB = QB // 128             # 128-k-tiles per query block
    NKT = S // 128             # k tiles per batch
    NMB = H // 512             # output-projection column blocks
    NT = TC // 128             # output-projection row tiles
    fp8 = compute_dtype == "fp8"
    CD = BF16                  # on-chip attention / v / out-proj dtype
    QKD = FP8 if fp8 else BF16  # q/k projection operand dtype
    ISQ = 1.0 / math.sqrt(HD)
    TI = 512                   # tokens per QKV iteration
    NIT = T // TI
    nh2 = HT // 2
    NGQ = NH // 4              # phase-3 head quads

    nc = bacc.Bacc("TRN2", target_bir_lowering=False, debug=False,
                   enable_asserts=True, num_devices=n_cores)

    # pre-tiled [block, 128, cols] operands (host prepares the layouts)
    hsTt = nc.declare_dram_parameter("hsTt", [NIT * 2, 128, nh2 * TI], CD,
                                     isOutput=False)
    if fp8:
        hsT8t = nc.declare_dram_parameter("hsT8t", [NIT * 2, 128, nh2 * TI],
                                          FP8, isOutput=False)
    wqTt = nc.declare_dram_parameter("wqTt", [128, HT * D], QKD, isOutput=False)
    wkTt = nc.declare_dram_parameter("wkTt", [128, HT * HD], QKD, isOutput=False)
    wvTt = nc.declare_dram_parameter("wvTt", [128, HT * HD], CD, isOutput=False)
    woTt = nc.declare_dram_parameter("woTt", [NMB // 2 * NGQ, 128, 4 * 1024],
                                     CD, isOutput=False)
    cosT = nc.declare_dram_parameter("cosT", [HD, S], F32, isOutput=False)
    sgnT = nc.declare_dram_parameter("sgnT", [HD, S], F32, isOutput=False)
    idn = nc.declare_dram_parameter("idn", [128, 128], CD, isOutput=False)
    if causal:
        dmask = nc.declare_dram_parameter("dmask", [128, KB * QB], CD, isOutput=False)
    else:
        maskT = nc.declare_dram_parameter("maskT", [S, S], F32, isOutput=False)
    out_c = nc.declare_dram_parameter("out", [TC, H], F32, isOutput=True)

    with tile.TileContext(nc) as tc:
        with tc.tile_pool(name="persist", bufs=1) as per, \
             tc.tile_pool(name="dram", bufs=1, space="DRAM") as dram:
            # persistent SBUF tensors
            qt = per.tile([128, QHC * T], CD)      # rope'd Q, head-major [d, t]
            kt = per.tile([128, T], CD)            # rope'd K [d, t]
            vt = per.tile([128, T], CD)            # V tiles [t(128), d] at col k*128
            cs = per.tile([128, S], F32)
            sg = per.tile([128, S], F32)
            ident = per.tile([128, 128], CD)
            ones_c = per.tile([128, 1], CD)        # den-matmul stationary
            ones_f32 = per.tile([1, 128], F32)
            ones_fr = per.tile([1, 128], F32R)     # bcast-matmul stationary
            nc.sync.dma_start(cs[:], cosT[:])
            nc.sync.dma_start(sg[:], sgnT[:])
            nc.sync.dma_start(ident[:], idn[:])
            nc.gpsimd.memset(ones_c[:], 1.0)
            nc.gpsimd.memset(ones_f32[:], 1.0)
            nc.scalar.activation(ones_fr[:], ones_f32[:], AF.Copy)
            # pre-warm the exp table set during the initial DMA wait so the
            # ~2.7us ACT_TABLE_LOAD is off the attention critical path
            warm = per.tile([1, 1], F32)
            nc.scalar.activation(warm[:], ones_f32[:, 0:1], AF.Exp)
            if causal:
                dm = per.tile([128, KB * QB], CD)
                nc.sync.dma_start(dm[:], dmask[:])

            # per-(head, batch) A2A bounce buffers; tokens interleaved by
            # batch so each batch ships in its own half-size collective
            a2a_in = [[dram.tile([n_cores, 128, TB], CD,
                                 name=f"a2a_in{i}_{b}") for b in range(B)]
                      for i in range(QHC)]
            a2a_out = [[dram.tile([n_cores, 128, TB], CD,
                                  name=f"a2a_out{i}_{b}") for b in range(B)]
                       for i in range(QHC)]

            # ---------------- Phase 1: QKV projections + RoPE ----------------
            with tc.tile_pool(name="wq", bufs=1) as wqp, \
                 tc.tile_pool(name="hst", bufs=3) as hstp, \
                 tc.tile_pool(name="qkps", bufs=2, space="PSUM") as qkps, \
                 tc.tile_pool(name="vps", bufs=2, space="PSUM") as vps, \
                 tc.tile_pool(name="epi", bufs=3) as epi:
                wq_sb = wqp.tile([128, HT * D], QKD)
                wk_sb = wqp.tile([128, HT * HD], QKD)
                wv_sb = wqp.tile([128, HT * HD], CD)
                # split the big weight loads so the first matmuls only wait
                # on their own quarter of the table
                wqc = HT * D // 4
                for q4 in range(4):
                    nc.sync.dma_start(wq_sb[:, q4 * wqc:(q4 + 1) * wqc],
                                      wqTt[:, q4 * wqc:(q4 + 1) * wqc])
                nc.sync.dma_start(wk_sb[:], wkTt[:])
                nc.sync.dma_start(wv_sb[:], wvTt[:])
                wq_r = wq_sb[:].rearrange("p (ht d) -> p ht d", ht=HT)
                wk_r = wk_sb[:].rearrange("p (ht d) -> p ht d", ht=HT)

                def rope(pp, dst_ap, sc):
                    # dst = ab*cos + rotate_half(ab)*sin with ab = pp (psum).
                    # The half-rotation crosses partitions, which compute
                    # engines cannot do SBUF->SBUF, so shift via on-chip DMA.
                    ab = epi.tile([128, TI], F32, tag="ab", name="ab")
                    nc.scalar.activation(ab[:], pp[:], AF.Copy)
                    sh = epi.tile([128, TI], F32, tag="sh", name="sh")
                    # scalar-engine HWDGE ring keeps these small shifts off
                    # the sync ring that streams the hs tiles
                    nc.scalar.dma_start(sh[0:64, :], ab[64:128, :])
                    nc.scalar.dma_start(sh[64:128, :], ab[0:64, :])
                    x1 = epi.tile([128, TI], F32, tag="x1", name="x1")
                    nc.vector.tensor_mul(x1[:], ab[:], cs[:, sc:sc + TI])
                    nc.vector.tensor_mul(sh[:], sh[:], sg[:, sc:sc + TI])
                    nc.vector.tensor_add(dst_ap, x1[:], sh[:])

                for it in range(NIT):
                    t0 = it * TI
                    sc = t0 % S  # column into cos/sgn tables
                    hsp8_halves = []
                    hsp_halves = []
                    # fp8 tiles load first: the q/k DoubleRow matmuls lead
                    # each iteration, the bf16 v matmuls trail it
                    if fp8:
                        for half in range(2):
                            hsp8 = hstp.tile([128, nh2 * TI], FP8, tag="hsp8",
                                             name=f"hsp8_{it}_{half}")
                            nc.sync.dma_start(hsp8[:], hsT8t[2 * it + half])
                            hsp8_halves.append(hsp8)
                    for half in range(2):
                        hsp = hstp.tile([128, nh2 * TI], CD, tag="hsp",
                                        name=f"hsp_{it}_{half}")
                        nc.sync.dma_start(hsp[:], hsTt[2 * it + half])
                        hsp_halves.append(hsp)

                    def hs_t(ht):
                        h = hsp_halves[ht // nh2]
                        j = ht % nh2
                        return h[:, j * TI:(j + 1) * TI]

                    def hs8_pair(j):
                        # [128, 2, TI] for DoubleRow (both ht in same half)
                        h = hsp8_halves[(2 * j) // nh2]
                        jj = (2 * j) % nh2
                        return h[:].rearrange("p (ht t) -> p ht t",
                                              ht=nh2)[:, jj:jj + 2, :]

                    for g in range(QHC + 1):  # 4 q heads, then k
                        pp = qkps.tile([128, TI], F32, tag="pp",
                                       name=f"pp_{it}_{g}")
                        if fp8:
                            for j in range(HT // 2):
                                if g < QHC:
                                    w_ap = wq_r[:, 2 * j:2 * j + 2,
                                                g * 128:(g + 1) * 128]
                                else:
                                    w_ap = wk_r[:, 2 * j:2 * j + 2, :]
                                nc.tensor.matmul(pp[:], w_ap, hs8_pair(j),
                                                 start=(j == 0),
                                                 stop=(j == HT // 2 - 1),
                                                 perf_mode=DR)
                        else:
                            for ht in range(HT):
                                if g < QHC:
                                    w_ap = wq_sb[:, ht * D + g * 128:
                                                 ht * D + (g + 1) * 128]
                                else:
                                    w_ap = wk_sb[:, ht * HD:(ht + 1) * HD]
                                nc.tensor.matmul(pp[:], w_ap, hs_t(ht),
                                                 start=(ht == 0),
                                                 stop=(ht == HT - 1))
                        if g < QHC:
                            rope(pp, qt[:, g * T + t0: g * T + t0 + TI], sc)
                        else:
                            rope(pp, kt[:, t0:t0 + TI], sc)
                    # v (always bf16): [d, t] accumulation, then transpose
                    pp = qkps.tile([128, TI], F32, tag="pp", name=f"ppv_{it}")
                    for ht in range(HT):
                        w_ap = wv_sb[:, ht * HD:(ht + 1) * HD]
                        nc.tensor.matmul(pp[:], w_ap, hs_t(ht),
                                         start=(ht == 0), stop=(ht == HT - 1))
                    vdt = epi.tile([128, TI], CD, tag="vdt", name="vdt")
                    nc.scalar.activation(vdt[:], pp[:], AF.Copy)
                    vtp = vps.tile([128, TI], CD, tag="vtp", name=f"vtp_{it}")
                    for tsub in range(TI // 128):
                        nc.tensor.transpose(
                            vtp[:, tsub * 128:(tsub + 1) * 128],
                            vdt[:, tsub * 128:(tsub + 1) * 128],
                            ident[:])
                    nc.vector.tensor_copy(vt[:, t0:t0 + TI], vtp[:])

            tc.no_sync_barrier()
            # ---------------- Phase 2: attention ----------------
            with tc.tile_pool(name="stps", bufs=2, space="PSUM") as stps, \
                 tc.tile_pool(name="otps", bufs=2, space="PSUM") as otps, \
                 tc.tile_pool(name="dbps", bufs=1, space="PSUM") as dbps, \
                 tc.tile_pool(name="att", bufs=6) as att, \
                 tc.tile_pool(name="attm", bufs=3) as attm, \
                 tc.tile_pool(name="accp", bufs=2) as accp:
                for hl in range(QHC):
                    for b in range(B):
                        for qb in range(NQB):
                            q0 = b * S + qb * QB          # global q col
                            n_k = (qb + 1) * KB if causal else NKT
                            otp = otps.tile([128, QB], F32, tag="ot",
                                            name=f"ot_{hl}_{b}_{qb}")
                            acc = accp.tile([128, 2 * QB], CD, tag="acc",
                                            name=f"acc_{hl}_{b}_{qb}")
                            for kp in range(n_k // 2):
                                stp = stps.tile([128, 2 * QB], F32, tag="st",
                                                name=f"st_{hl}_{b}_{qb}_{kp}")
                                for half in range(2):
                                    kti = 2 * kp + half
                                    kg = b * NKT + kti
                                    nc.tensor.matmul(
                                        stp[:, half * QB:(half + 1) * QB],
                                        kt[:, kg * 128:(kg + 1) * 128],
                                        qt[:, hl * T + q0: hl * T + q0 + QB],
                                        start=True, stop=True)
                                pt = att.tile([128, 2 * QB], CD, tag="pt",
                                              name=f"pt_{hl}_{b}_{qb}_{kp}")
                                d0 = 2 * kp - qb * KB  # diag pattern index
                                if causal and 2 * kp + 1 >= qb * KB:
                                    nc.scalar.activation(pt[:], stp[:], AF.Exp,
                                                         scale=ISQ)
                                    nc.vector.tensor_mul(
                                        pt[:], pt[:],
                                        dm[:, d0 * QB:(d0 + 2) * QB])
                                elif not causal:
                                    mt = attm.tile([128, 2 * QB], F32, tag="mt",
                                                   name="mt")
                                    for half in range(2):
                                        kti = 2 * kp + half
                                        nc.sync.dma_start(
                                            mt[:, half * QB:(half + 1) * QB],
                                            maskT[kti * 128:(kti + 1) * 128,
                                                  qb * QB:(qb + 1) * QB])
                                    tmp = att.tile([128, 2 * QB], F32, tag="tmp",
                                                   name="tmp")
                                    nc.vector.tensor_add(tmp[:], stp[:], mt[:])
                                    nc.scalar.activation(pt[:], tmp[:], AF.Exp,
                                                         scale=ISQ)
                                else:
                                    nc.scalar.activation(pt[:], stp[:], AF.Exp,
                                                         scale=ISQ)
                                for half in range(2):
                                    kti = 2 * kp + half
                                    kg = b * NKT + kti
                                    nc.tensor.matmul(
                                        otp[:], vt[:, kg * 128:(kg + 1) * 128],
                                        pt[:, half * QB:(half + 1) * QB],
                                        start=(kti == 0), stop=(kti == n_k - 1))
                                if kp == 0:
                                    nc.vector.tensor_copy(acc[:], pt[:])
                                else:
                                    nc.vector.tensor_add(acc[:], acc[:], pt[:])
                            # denominator: partition-reduce the acc chain on
                            # PE, bcast, fast reciprocal, normalize
                            db = dbps.tile([1, QB], F32, tag="db",
                                           name=f"db_{hl}_{b}_{qb}")
                            nc.tensor.matmul(db[0:1, :], ones_c[:],
                                             acc[:, 0:QB],
                                             start=True, stop=False)
                            nc.tensor.matmul(db[0:1, :], ones_c[:],
                                             acc[:, QB:2 * QB],
                                             start=False, stop=True)
                            dsb = att.tile([1, QB], F32R, tag="dsb", name="dsb")
                            nc.vector.tensor_copy(dsb[:], db[0:1, :])
                            dbc = dbps.tile([128, QB], F32, tag="dbc",
                                            name=f"dbc_{hl}_{b}_{qb}")
                            nc.tensor.matmul(dbc[:], ones_fr[:], dsb[:],
                                             start=True, stop=True)
                            rec = att.tile([128, QB], F32, tag="rec", name="rec")
                            nc.vector.reciprocal_approx_fast(rec[:], dbc[:])
                            ot_sb = att.tile([128, QB], CD, tag="otsb",
                                             name="otsb")
                            nc.vector.tensor_mul(ot_sb[:], otp[:], rec[:])
                            # scatter into this (head, batch) A2A buffer
                            tb0 = qb * QB
                            nj = max(1, QB // TB)
                            j0 = tb0 // TB
                            c0 = tb0 % TB
                            if nj == 1:
                                nc.sync.dma_start(
                                    a2a_in[hl][b][j0, :, c0:c0 + QB], ot_sb[:])
                            else:
                                for j in range(nj):
                                    nc.sync.dma_start(
                                        a2a_in[hl][b][j0 + j, :, :],
                                        ot_sb[:, j * TB:(j + 1) * TB])
                        nc.gpsimd.collective_compute(
                            "AllToAll", ALU.bypass,
                            replica_groups=[list(range(n_cores))],
                            ins=[a2a_in[hl][b][:]],
                            outs=[a2a_out[hl][b][:]])

            # no barrier: phase 3's gathers and weight loads overlap the
            # attention tail; its matmuls wait on PSUM release + data deps
            # ---------------- Phase 3: output projection ----------------
            with tc.tile_pool(name="otl", bufs=1) as otlp, \
                 tc.tile_pool(name="wot", bufs=6) as wotp, \
                 tc.tile_pool(name="ops", bufs=1, space="PSUM") as ops, \
                 tc.tile_pool(name="oout", bufs=6) as ooutp:
                ot_loc = otlp.tile([128, NH * TC], CD)
                d_order = [i * QHC + hl for hl in range(QHC) for i in range(n_cores)]
                for g in d_order:
                    for b in range(B):
                        nc.scalar.dma_start(
                            ot_loc[:, g * TC + b * TB:g * TC + (b + 1) * TB],
                            a2a_out[g % QHC][b][g // QHC, :, :])
                for mp in range(NMB // 2):
                    pos = [ops.tile([128, 512], F32, tag=f"po{sub}{tt}",
                                    name=f"po_{mp}_{sub}_{tt}")
                           for sub in range(2) for tt in range(NT)]
                    for gq in range(NGQ):
                        # pre-tiled quad of four heads' wo rows
                        wot = wotp.tile([128, 4 * 1024], CD, tag="wot",
                                        name=f"wot_{mp}_{gq}")
                        nc.sync.dma_start(wot[:], woTt[mp * NGQ + gq])
                        for gi in range(4):
                            g = d_order[gq * 4 + gi]
                            first = gq == 0 and gi == 0
                            last = gq == NGQ - 1 and gi == 3
                            for sub in range(2):
                                for tt in range(NT):
                                    nc.tensor.matmul(
                                        pos[sub * NT + tt],
                                        ot_loc[:, g * TC + tt * 128:
                                               g * TC + (tt + 1) * 128],
                                        wot[:, gi * 1024 + sub * 512:
                                            gi * 1024 + (sub + 1) * 512],
                                        start=first, stop=last)
                    for sub in range(2):
                        for tt in range(NT):
                            ob = ooutp.tile([128, 512], F32, tag="ob", name="ob")
                            nc.scalar.activation(ob[:], pos[sub * NT + tt],
                                                 AF.Copy)
                            nc.scalar.dma_start(
                                out_c[tt * 128:(tt + 1) * 128,
                                      (2 * mp + sub) * 512:(2 * mp + sub + 1) * 512],
                                ob[:])

    nc.compile()
    return nc


def _np16(x):
    return np.asarray(x, dtype=ml_dtypes.bfloat16)


def _np8(x):
    return np.asarray(np.clip(np.asarray(x, np.float32), -224.0, 224.0),
                      dtype=ml_dtypes.float8_e4m3)


def _tile_hs(hsT, NIT, TI, nh2):
    """[H, T] -> [NIT*2, 128, nh2*TI] matching the SBUF hsp layout."""
    H, T = hsT.shape
    x = hsT.reshape(2, nh2, 128, NIT, TI)
    x = np.transpose(x, (3, 0, 2, 1, 4))  # [NIT, half, p, ht, t]
    return np.ascontiguousarray(x.reshape(NIT * 2, 128, nh2 * TI))


def _tile_w(wT, HT, D):
    """[H, D] -> [128, HT*D] matching the SBUF weight layout."""
    return np.ascontiguousarray(
        wT.reshape(HT, 128, D).transpose(1, 0, 2).reshape(128, HT * D))


def _tile_wo(woT, d_order, NMB, NGQ):
    """[NH*HD, H] -> [NMB/2*NGQ, 128, 4096] per-(mp, quad) blocks."""
    blocks = np.empty((NMB // 2 * NGQ, 128, 4096), dtype=woT.dtype)
    for mp in range(NMB // 2):
        for gq in range(NGQ):
            for gi in range(4):
                g = d_order[gq * 4 + gi]
                blocks[mp * NGQ + gq, :, gi * 1024:(gi + 1) * 1024] = \
                    woT[g * 128:(g + 1) * 128, mp * 1024:(mp + 1) * 1024]
    return blocks


def prep_inputs(hidden_states, attention_mask, cos, sin, wq, wk, wv, wo,
                compute_dtype="fp8", n_cores=N_CORES):
    """Host-side sharding + pre-tiling. Returns (in_maps, causal, dims)."""
    B, S, H = hidden_states.shape
    T = B * S
    D = NH * HD // n_cores
    KD = NKV * HD // n_cores
    HT = H // 128
    TI = 512
    NIT = T // TI
    nh2 = HT // 2
    NMB = H // 512
    NGQ = NH // 4
    QHC = NH // n_cores
    fp8 = compute_dtype == "fp8"

    hs2 = np.asarray(hidden_states, np.float32).reshape(T, H)
    hsTt = _tile_hs(_np16(hs2.T), NIT, TI, nh2)
    d_order = [i * QHC + hl for hl in range(QHC) for i in range(n_cores)]
    woTt = _tile_wo(_np16(np.asarray(wo, np.float32).T), d_order, NMB, NGQ)
    rope_scale = (1.0 / S_QKV) if fp8 else 1.0
    cosT = np.ascontiguousarray(np.asarray(cos, np.float32)[0].T * rope_scale)
    sinT = np.ascontiguousarray(np.asarray(sin, np.float32)[0].T) * rope_scale
    sgnT = np.concatenate([-sinT[0:HD // 2], sinT[HD // 2:]], axis=0)
    sgnT = np.ascontiguousarray(sgnT)

    m = np.asarray(attention_mask, np.float32)[0, 0]
    expected = np.where(np.tril(np.ones((S, S), bool)), 0.0, NEG).astype(np.float32)
    causal = bool(np.array_equal(m, expected))

    TC = T // n_cores
    QB = min(512, TC)
    KB = QB // 128
    qkd = (lambda x: _np8(x * S_W)) if fp8 else (lambda x: _np16(x))
    in_maps = []
    for c in range(n_cores):
        im = {
            "hsTt": hsTt,
            "wqTt": _tile_w(qkd(np.asarray(wq, np.float32)[c * D:(c + 1) * D].T),
                            HT, D),
            "wkTt": _tile_w(qkd(np.asarray(wk, np.float32)[c * KD:(c + 1) * KD].T),
                            HT, HD),
            "wvTt": _tile_w(_np16(np.asarray(wv, np.float32)[c * KD:(c + 1) * KD].T),
                            HT, HD),
            "woTt": woTt,
            "cosT": cosT,
            "sgnT": sgnT,
            "idn": _np16(np.eye(128, dtype=np.float32)),
        }
        if fp8:
            im["hsT8t"] = _tile_hs(_np8(hs2.T * S_HS), NIT, TI, nh2)
        if causal:
            pk = np.arange(128)[:, None]
            pq = np.arange(QB)[None, :]
            dmask = np.concatenate(
                [np.where(pk + j * 128 <= pq, 1.0, 0.0) for j in range(KB)],
                axis=1).astype(np.float32)
            im["dmask"] = _np16(dmask)
        else:
            im["maskT"] = np.ascontiguousarray(m.T)
        in_maps.append(im)
    return in_maps, causal, (T, S, H)


_BUILD_CACHE = {}


def kernel(hidden_states, attention_mask, cos, sin, wq, wk, wv, wo,
           compute_dtype="fp8", trace=False):
    B, S, H = hidden_states.shape
    T = B * S
    in_maps, causal, dims = prep_inputs(
        hidden_states, attention_mask, cos, sin, wq, wk, wv, wo,
        compute_dtype=compute_dtype)
    key = (T, S, H, compute_dtype, causal)
    if key not in _BUILD_CACHE:
        _BUILD_CACHE[key] = build(T, S, H, compute_dtype=compute_dtype,
                                  causal=causal)
    nc = _BUILD_CACHE[key]
    res = run_bass_kernel_spmd(nc, in_maps, core_ids=list(range(N_CORES)),
                               trace=trace)
    TC = T // N_CORES
    TB = TC // B
    out = np.empty((T, H), np.float32)
    for c in range(N_CORES):
        oc = res.results[c]["out"]
        for b in range(B):
            out[b * S + c * TB: b * S + (c + 1) * TB] = \
                oc[b * TB:(b + 1) * TB]
    if trace:
        kernel.last_exec_time_ns = res.exec_time_ns
        kernel.last_results = res
    return out.reshape(B, S, H)


# revision 31
# speedup vs baseline: 1.0776x; 1.0764x over previous
"""Fused Mllama-style text self-attention on one TRN2 chip (8 NeuronCores).

Sharding: tensor-parallel over heads (4 q heads / 1 kv head per core) for the
QKV projections + RoPE + attention; per-(head, batch) AllToAlls reshard the
attention outputs to token-parallel (tokens interleaved across cores by
batch), so each core computes a 512-token slice of the final output
projection against the full wo.

Precision strategy: the Q/K projections run in fp8e4 with DoubleRow matmuls
(scores feed a softmax whose inputs are tiny, so score error washes out);
V, attention, and the output projection stay bf16.

All large DRAM operands are pre-tiled on the host so that every SBUF load is
a single contiguous [128, N] DMA.

kernel(**inputs) takes the FULL (unsharded) inputs and returns the FULL output.
"""

import math

import numpy as np
import ml_dtypes

import concourse.bacc as bacc
import concourse.bass as bass
import concourse.mybir as mybir
import concourse.tile as tile
from concourse.bass_utils import run_bass_kernel_spmd

F32 = mybir.dt.float32
F32R = mybir.dt.float32r
BF16 = mybir.dt.bfloat16
FP8 = mybir.dt.float8e4
AF = mybir.ActivationFunctionType
ALU = mybir.AluOpType
DR = mybir.MatmulPerfMode.DoubleRow

NH, NKV, HD = 32, 8, 128
NEG = -1.0e9
N_CORES = 8

# fp8 scaling for the q/k projections: host multiplies hs and wq/wk by 32;
# the rope cos/sin tables carry the 1/1024 correction.
S_HS = 32.0
S_W = 32.0
S_QKV = S_HS * S_W


def build(T, S, H, compute_dtype="fp8", causal=True, n_cores=N_CORES):
    """Build the SPMD Bass program (same program for all cores)."""
    B = T // S
    TC = T // n_cores          # tokens per core in the output projection
    TB = TC // B               # tokens per (core, batch)
    QHC = NH // n_cores        # local q heads (4)
    D = QHC * HD               # local q width (512)
    HT = H // 128              # contraction tiles over hidden
    QB = min(512, TC)          # attention query block width
    NQB = S // QB              # query blocks per batch
    KB = QB // 128             # 128-k-tiles per query block
    NKT = S // 128             # k tiles per batch
    NMB = H // 512             # output-projection column blocks
    NT = TC // 128             # output-projection row tiles
    fp8 = compute_dtype == "fp8"
    CD = BF16                  # on-chip attention / v / out-proj dtype
    QKD = FP8 if fp8 else BF16  # q/k projection operand dtype
    ISQ = 1.0 / math.sqrt(HD)
    TI = 512                   # tokens per QKV iteration
    NIT = T // TI
    nh2 = HT // 2
    NGQ = NH // 4              # phase-3 head quads

    nc = bacc.Bacc("TRN2", target_bir_lowering=False, debug=False,
                   enable_asserts=True, num_devices=n_cores)

    # pre-tiled [block, 128, cols] operands (host prepares the layouts)
    hsTt = nc.declare_dram_parameter("hsTt", [NIT * 2, 128, nh2 * TI], CD,
                                     isOutput=False)
    if fp8:
        hsT8t = nc.declare_dram_parameter("hsT8t", [NIT * 2, 128, nh2 * TI],
                                          FP8, isOutput=False)
    wqTt = nc.declare_dram_parameter("wqTt", [128, HT * D], QKD, isOutput=False)
    wkTt = nc.declare_dram_parameter("wkTt", [128, HT * HD], QKD, isOutput=False)
    wvTt = nc.declare_dram_parameter("wvTt", [128, HT * HD], CD, isOutput=False)
    woTt = nc.declare_dram_parameter("woTt", [NMB // 2 * NGQ, 128, 4 * 1024],
                                     CD, isOutput=False)
    cosT = nc.declare_dram_parameter("cosT", [HD, S], F32, isOutput=False)
    sgnT = nc.declare_dram_parameter("sgnT", [HD, S], F32, isOutput=False)
    idn = nc.declare_dram_parameter("idn", [128, 128], CD, isOutput=False)
    if causal:
        dmask = nc.declare_dram_parameter("dmask", [128, KB * QB], CD, isOutput=False)
    else:
        maskT = nc.declare_dram_parameter("maskT", [S, S], F32, isOutput=False)
    out_c = nc.declare_dram_parameter("out", [TC, H], F32, isOutput=True)

    with tile.TileContext(nc) as tc:
        with tc.tile_pool(name="persist", bufs=1) as per, \
             tc.tile_pool(name="dram", bufs=1, space="DRAM") as dram:
            # persistent SBUF tensors
            qt = per.tile([128, QHC * T], CD)      # rope'd Q, head-major [d, t]
            kt = per.tile([128, T], CD)            # rope'd K [d, t]
            vt = per.tile([128, T], CD)            # V tiles [t(128), d] at col k*128
            cs = per.tile([128, S], F32)
            sg = per.tile([128, S], F32)
            ident = per.tile([128, 128], CD)
            ones_c = per.tile([128, 1], CD)        # den-matmul stationary
            ones_f32 = per.tile([1, 128], F32)
            ones_fr = per.tile([1, 128], F32R)     # bcast-matmul stationary
            nc.gpsimd.memset(ones_c[:], 1.0)
            nc.gpsimd.memset(ones_f32[:], 1.0)
            nc.scalar.activation(ones_fr[:], ones_f32[:], AF.Copy)
            # pre-warm the exp table set during the initial DMA wait so the
            # ~2.7us ACT_TABLE_LOAD is off the attention critical path
            warm = per.tile([1, 1], F32)
            nc.scalar.activation(warm[:], ones_f32[:, 0:1], AF.Exp)
            if causal:
                dm = per.tile([128, KB * QB], CD)

            # per-(head, batch) A2A bounce buffers; tokens interleaved by
            # batch so each batch ships in its own half-size collective
            a2a_in = [[dram.tile([n_cores, 128, TB], CD,
                                 name=f"a2a_in{i}_{b}") for b in range(B)]
                      for i in range(QHC)]
            a2a_out = [[dram.tile([n_cores, 128, TB], CD,
                                  name=f"a2a_out{i}_{b}") for b in range(B)]
                       for i in range(QHC)]

            # ---------------- Phase 1: QKV projections + RoPE ----------------
            with tc.tile_pool(name="wq", bufs=1) as wqp, \
                 tc.tile_pool(name="hst", bufs=3) as hstp, \
                 tc.tile_pool(name="qkps", bufs=2, space="PSUM") as qkps, \
                 tc.tile_pool(name="vps", bufs=2, space="PSUM") as vps, \
                 tc.tile_pool(name="epi", bufs=3) as epi:
                wq_sb = wqp.tile([128, HT * D], QKD)
                wk_sb = wqp.tile([128, HT * HD], QKD)
                wv_sb = wqp.tile([128, HT * HD], CD)
                # wq is laid out HEAD-major; the first head's block loads
                # first so the first matmul group waits on 0.5MB, not 2MB.
                # Remaining loads are sequenced on the sync-ring FIFO between
                # the critical tiles, each arriving just before first use.
                HB = HT * 128
                nc.sync.dma_start(wq_sb[:, 0:HB], wqTt[:, 0:HB])
                wk_r = wk_sb[:].rearrange("p (ht d) -> p ht d", ht=HT)

                def wq_g(g):
                    return wq_sb[:, g * HB:(g + 1) * HB].rearrange(
                        "p (ht d) -> p ht d", ht=HT)

                def rope(pp, dst_ap, sc):
                    # dst = ab*cos + rotate_half(ab)*sin with ab = pp (psum).
                    # The half-rotation crosses partitions, which compute
                    # engines cannot do SBUF->SBUF, so shift via on-chip DMA.
                    ab = epi.tile([128, TI], F32, tag="ab", name="ab")
                    nc.scalar.activation(ab[:], pp[:], AF.Copy)
                    sh = epi.tile([128, TI], F32, tag="sh", name="sh")
                    # scalar-engine HWDGE ring keeps these small shifts off
                    # the sync ring that streams the hs tiles
                    nc.scalar.dma_start(sh[0:64, :], ab[64:128, :])
                    nc.scalar.dma_start(sh[64:128, :], ab[0:64, :])
                    x1 = epi.tile([128, TI], F32, tag="x1", name="x1")
                    nc.vector.tensor_mul(x1[:], ab[:], cs[:, sc:sc + TI])
                    nc.vector.tensor_mul(sh[:], sh[:], sg[:, sc:sc + TI])
                    nc.vector.tensor_add(dst_ap, x1[:], sh[:])

                for it in range(NIT):
                    t0 = it * TI
                    sc = t0 % S  # column into cos/sgn tables
                    hsp8_halves = []
                    hsp_halves = []
                    # fp8 tiles load first: the q/k DoubleRow matmuls lead
                    # each iteration, the bf16 v matmuls trail it
                    if fp8:
                        for half in range(2):
                            hsp8 = hstp.tile([128, nh2 * TI], FP8, tag="hsp8",
                                             name=f"hsp8_{it}_{half}")
                            nc.sync.dma_start(hsp8[:], hsT8t[2 * it + half])
                            hsp8_halves.append(hsp8)
                    if it == 0:
                        # staged behind the first critical tiles, ahead of use
                        nc.sync.dma_start(wq_sb[:, HB:2 * HB], wqTt[:, HB:2 * HB])
                        nc.sync.dma_start(cs[:], cosT[:])
                        nc.sync.dma_start(sg[:], sgnT[:])
                        nc.sync.dma_start(wq_sb[:, 2 * HB:3 * HB],
                                          wqTt[:, 2 * HB:3 * HB])
                        nc.sync.dma_start(wq_sb[:, 3 * HB:4 * HB],
                                          wqTt[:, 3 * HB:4 * HB])
                        nc.sync.dma_start(wk_sb[:], wkTt[:])
                    for half in range(2):
                        hsp = hstp.tile([128, nh2 * TI], CD, tag="hsp",
                                        name=f"hsp_{it}_{half}")
                        nc.sync.dma_start(hsp[:], hsTt[2 * it + half])
                        hsp_halves.append(hsp)
                    if it == 0:
                        nc.sync.dma_start(wv_sb[:], wvTt[:])
                        nc.sync.dma_start(ident[:], idn[:])
                        if causal:
                            nc.sync.dma_start(dm[:], dmask[:])

                    def hs_t(ht):
                        h = hsp_halves[ht // nh2]
                        j = ht % nh2
                        return h[:, j * TI:(j + 1) * TI]

                    def hs8_pair(j):
                        # [128, 2, TI] for DoubleRow (both ht in same half)
                        h = hsp8_halves[(2 * j) // nh2]
                        jj = (2 * j) % nh2
                        return h[:].rearrange("p (ht t) -> p ht t",
                                              ht=nh2)[:, jj:jj + 2, :]

                    for g in range(QHC + 1):  # 4 q heads, then k
                        pp = qkps.tile([128, TI], F32, tag="pp",
                                       name=f"pp_{it}_{g}")
                        if fp8:
                            for j in range(HT // 2):
                                if g < QHC:
                                    w_ap = wq_g(g)[:, 2 * j:2 * j + 2, :]
                                else:
                                    w_ap = wk_r[:, 2 * j:2 * j + 2, :]
                                nc.tensor.matmul(pp[:], w_ap, hs8_pair(j),
                                                 start=(j == 0),
                                                 stop=(j == HT // 2 - 1),
                                                 perf_mode=DR)
                        else:
                            for ht in range(HT):
                                if g < QHC:
                                    w_ap = wq_sb[:, g * HB + ht * 128:
                                                 g * HB + (ht + 1) * 128]
                                else:
                                    w_ap = wk_sb[:, ht * HD:(ht + 1) * HD]
                                nc.tensor.matmul(pp[:], w_ap, hs_t(ht),
                                                 start=(ht == 0),
                                                 stop=(ht == HT - 1))
                        if g < QHC:
                            rope(pp, qt[:, g * T + t0: g * T + t0 + TI], sc)
                        else:
                            rope(pp, kt[:, t0:t0 + TI], sc)
                    # v (always bf16): [d, t] accumulation, then transpose
                    pp = qkps.tile([128, TI], F32, tag="pp", name=f"ppv_{it}")
                    for ht in range(HT):
                        w_ap = wv_sb[:, ht * HD:(ht + 1) * HD]
                        nc.tensor.matmul(pp[:], w_ap, hs_t(ht),
                                         start=(ht == 0), stop=(ht == HT - 1))
                    vdt = epi.tile([128, TI], CD, tag="vdt", name="vdt")
                    nc.scalar.activation(vdt[:], pp[:], AF.Copy)
                    vtp = vps.tile([128, TI], CD, tag="vtp", name=f"vtp_{it}")
                    for tsub in range(TI // 128):
                        nc.tensor.transpose(
                            vtp[:, tsub * 128:(tsub + 1) * 128],
                            vdt[:, tsub * 128:(tsub + 1) * 128],
                            ident[:])
                    nc.vector.tensor_copy(vt[:, t0:t0 + TI], vtp[:])

            tc.no_sync_barrier()
            # ---------------- Phase 2: attention ----------------
            with tc.tile_pool(name="stps", bufs=2, space="PSUM") as stps, \
                 tc.tile_pool(name="otps", bufs=2, space="PSUM") as otps, \
                 tc.tile_pool(name="dbps", bufs=1, space="PSUM") as dbps, \
                 tc.tile_pool(name="att", bufs=6) as att, \
                 tc.tile_pool(name="attm", bufs=3) as attm, \
                 tc.tile_pool(name="accp", bufs=2) as accp:
                for hl in range(QHC):
                    for b in range(B):
                        for qb in range(NQB):
                            q0 = b * S + qb * QB          # global q col
                            n_k = (qb + 1) * KB if causal else NKT
                            otp = otps.tile([128, QB], F32, tag="ot",
                                            name=f"ot_{hl}_{b}_{qb}")
                            acc = accp.tile([128, 2 * QB], CD, tag="acc",
                                            name=f"acc_{hl}_{b}_{qb}")
                            for kp in range(n_k // 2):
                                stp = stps.tile([128, 2 * QB], F32, tag="st",
                                                name=f"st_{hl}_{b}_{qb}_{kp}")
                                for half in range(2):
                                    kti = 2 * kp + half
                                    kg = b * NKT + kti
                                    # causal-diagonal tiles: columns q <
                                    # (kti-qb*KB)*128 are fully masked — skip
                                    # them (qb 0 stays full-width so every
                                    # PSUM slot is initialized by a full
                                    # write before any partial one)
                                    qo = 0
                                    if causal and qb > 0 and kti >= qb * KB:
                                        qo = (kti - qb * KB) * 128
                                    nc.tensor.matmul(
                                        stp[:, half * QB + qo:(half + 1) * QB],
                                        kt[:, kg * 128:(kg + 1) * 128],
                                        qt[:, hl * T + q0 + qo:
                                           hl * T + q0 + QB],
                                        start=True, stop=True)
                                pt = att.tile([128, 2 * QB], CD, tag="pt",
                                              name=f"pt_{hl}_{b}_{qb}_{kp}")
                                d0 = 2 * kp - qb * KB  # diag pattern index
                                if causal and 2 * kp + 1 >= qb * KB:
                                    nc.scalar.activation(pt[:], stp[:], AF.Exp,
                                                         scale=ISQ)
                                    nc.vector.tensor_mul(
                                        pt[:], pt[:],
                                        dm[:, d0 * QB:(d0 + 2) * QB])
                                elif not causal:
                                    mt = attm.tile([128, 2 * QB], F32, tag="mt",
                                                   name="mt")
                                    for half in range(2):
                                        kti = 2 * kp + half
                                        nc.sync.dma_start(
                                            mt[:, half * QB:(half + 1) * QB],
                                            maskT[kti * 128:(kti + 1) * 128,
                                                  qb * QB:(qb + 1) * QB])
                                    tmp = att.tile([128, 2 * QB], F32, tag="tmp",
                                                   name="tmp")
                                    nc.vector.tensor_add(tmp[:], stp[:], mt[:])
                                    nc.scalar.activation(pt[:], tmp[:], AF.Exp,
                                                         scale=ISQ)
                                else:
                                    nc.scalar.activation(pt[:], stp[:], AF.Exp,
                                                         scale=ISQ)
                                for half in range(2):
                                    kti = 2 * kp + half
                                    kg = b * NKT + kti
                                    qo = 0
                                    if causal and qb > 0 and kti >= qb * KB:
                                        qo = (kti - qb * KB) * 128
                                    nc.tensor.matmul(
                                        otp[:, qo:QB],
                                        vt[:, kg * 128:(kg + 1) * 128],
                                        pt[:, half * QB + qo:(half + 1) * QB],
                                        start=(kti == 0), stop=(kti == n_k - 1))
                                if kp == 0:
                                    nc.vector.tensor_copy(acc[:], pt[:])
                                else:
                                    nc.vector.tensor_add(acc[:], acc[:], pt[:])
                            # denominator: partition-reduce the acc chain on
                            # PE, bcast, fast reciprocal, normalize
                            db = dbps.tile([1, QB], F32, tag="db",
                                           name=f"db_{hl}_{b}_{qb}")
                            nc.tensor.matmul(db[0:1, :], ones_c[:],
                                             acc[:, 0:QB],
                                             start=True, stop=False)
                            nc.tensor.matmul(db[0:1, :], ones_c[:],
                                             acc[:, QB:2 * QB],
                                             start=False, stop=True)
                            dsb = att.tile([1, QB], F32R, tag="dsb", name="dsb")
                            nc.vector.tensor_copy(dsb[:], db[0:1, :])
                            dbc = dbps.tile([128, QB], F32, tag="dbc",
                                            name=f"dbc_{hl}_{b}_{qb}")
                            nc.tensor.matmul(dbc[:], ones_fr[:], dsb[:],
                                             start=True, stop=True)
                            rec = att.tile([128, QB], F32, tag="rec", name="rec")
                            nc.vector.reciprocal_approx_fast(rec[:], dbc[:])
                            ot_sb = att.tile([128, QB], CD, tag="otsb",
                                             name="otsb")
                            nc.vector.tensor_mul(ot_sb[:], otp[:], rec[:])
                            # scatter into this (head, batch) A2A buffer
                            tb0 = qb * QB
                            nj = max(1, QB // TB)
                            j0 = tb0 // TB
                            c0 = tb0 % TB
                            if nj == 1:
                                nc.sync.dma_start(
                                    a2a_in[hl][b][j0, :, c0:c0 + QB], ot_sb[:])
                            else:
                                for j in range(nj):
                                    nc.sync.dma_start(
                                        a2a_in[hl][b][j0 + j, :, :],
                                        ot_sb[:, j * TB:(j + 1) * TB])
                        nc.gpsimd.collective_compute(
                            "AllToAll", ALU.bypass,
                            replica_groups=[list(range(n_cores))],
                            ins=[a2a_in[hl][b][:]],
                            outs=[a2a_out[hl][b][:]])

            # no barrier: phase 3's gathers and weight loads overlap the
            # attention tail; its matmuls wait on PSUM release + data deps
            # ---------------- Phase 3: output projection ----------------
            with tc.tile_pool(name="otl", bufs=1) as otlp, \
                 tc.tile_pool(name="wot", bufs=6) as wotp, \
                 tc.tile_pool(name="ops", bufs=1, space="PSUM") as ops, \
                 tc.tile_pool(name="oout", bufs=6) as ooutp:
                ot_loc = otlp.tile([128, NH * TC], CD)
                d_order = [i * QHC + hl for hl in range(QHC) for i in range(n_cores)]
                for g in d_order:
                    for b in range(B):
                        nc.scalar.dma_start(
                            ot_loc[:, g * TC + b * TB:g * TC + (b + 1) * TB],
                            a2a_out[g % QHC][b][g // QHC, :, :])
                for mp in range(NMB // 2):
                    pos = [ops.tile([128, 512], F32, tag=f"po{sub}{tt}",
                                    name=f"po_{mp}_{sub}_{tt}")
                           for sub in range(2) for tt in range(NT)]
                    for gq in range(NGQ):
                        # pre-tiled quad of four heads' wo rows
                        wot = wotp.tile([128, 4 * 1024], CD, tag="wot",
                                        name=f"wot_{mp}_{gq}")
                        nc.sync.dma_start(wot[:], woTt[mp * NGQ + gq])
                        for gi in range(4):
                            g = d_order[gq * 4 + gi]
                            first = gq == 0 and gi == 0
                            last = gq == NGQ - 1 and gi == 3
                            for sub in range(2):
                                for tt in range(NT):
                                    nc.tensor.matmul(
                                        pos[sub * NT + tt],
                                        ot_loc[:, g * TC + tt * 128:
                                               g * TC + (tt + 1) * 128],
                                        wot[:, gi * 1024 + sub * 512:
                                            gi * 1024 + (sub + 1) * 512],
                                        start=first, stop=last)
                    for sub in range(2):
                        for tt in range(NT):
                            ob = ooutp.tile([128, 512], F32, tag="ob", name="ob")
                            nc.scalar.activation(ob[:], pos[sub * NT + tt],
                                                 AF.Copy)
                            nc.scalar.dma_start(
                                out_c[tt * 128:(tt + 1) * 128,
                                      (2 * mp + sub) * 512:(2 * mp + sub + 1) * 512],
                                ob[:])

    nc.compile()
    return nc


def _np16(x):
    return np.asarray(x, dtype=ml_dtypes.bfloat16)


def _np8(x):
    return np.asarray(np.clip(np.asarray(x, np.float32), -224.0, 224.0),
                      dtype=ml_dtypes.float8_e4m3)


def _tile_hs(hsT, NIT, TI, nh2):
    """[H, T] -> [NIT*2, 128, nh2*TI] matching the SBUF hsp layout."""
    H, T = hsT.shape
    x = hsT.reshape(2, nh2, 128, NIT, TI)
    x = np.transpose(x, (3, 0, 2, 1, 4))  # [NIT, half, p, ht, t]
    return np.ascontiguousarray(x.reshape(NIT * 2, 128, nh2 * TI))


def _tile_w(wT, HT, D):
    """[H, D] -> [128, HT*D] matching the SBUF weight layout."""
    return np.ascontiguousarray(
        wT.reshape(HT, 128, D).transpose(1, 0, 2).reshape(128, HT * D))


def _tile_w_hm(wT, HT, D):
    """[H, D] -> [128, D*HT] head-major (128-wide head blocks contiguous)."""
    nh = D // 128
    x = wT.reshape(HT, 128, nh, 128).transpose(1, 2, 0, 3)  # [p, g, ht, d]
    return np.ascontiguousarray(x.reshape(128, HT * D))


def _tile_wo(woT, d_order, NMB, NGQ):
    """[NH*HD, H] -> [NMB/2*NGQ, 128, 4096] per-(mp, quad) blocks."""
    blocks = np.empty((NMB // 2 * NGQ, 128, 4096), dtype=woT.dtype)
    for mp in range(NMB // 2):
        for gq in range(NGQ):
            for gi in range(4):
                g = d_order[gq * 4 + gi]
                blocks[mp * NGQ + gq, :, gi * 1024:(gi + 1) * 1024] = \
                    woT[g * 128:(g + 1) * 128, mp * 1024:(mp + 1) * 1024]
    return blocks


def prep_inputs(hidden_states, attention_mask, cos, sin, wq, wk, wv, wo,
                compute_dtype="fp8", n_cores=N_CORES):
    """Host-side sharding + pre-tiling. Returns (in_maps, causal, dims)."""
    B, S, H = hidden_states.shape
    T = B * S
    D = NH * HD // n_cores
    KD = NKV * HD // n_cores
    HT = H // 128
    TI = 512
    NIT = T // TI
    nh2 = HT // 2
    NMB = H // 512
    NGQ = NH // 4
    QHC = NH // n_cores
    fp8 = compute_dtype == "fp8"

    hs2 = np.asarray(hidden_states, np.float32).reshape(T, H)
    hsTt = _tile_hs(_np16(hs2.T), NIT, TI, nh2)
    d_order = [i * QHC + hl for hl in range(QHC) for i in range(n_cores)]
    woTt = _tile_wo(_np16(np.asarray(wo, np.float32).T), d_order, NMB, NGQ)
    rope_scale = (1.0 / S_QKV) if fp8 else 1.0
    cosT = np.ascontiguousarray(np.asarray(cos, np.float32)[0].T * rope_scale)
    sinT = np.ascontiguousarray(np.asarray(sin, np.float32)[0].T) * rope_scale
    sgnT = np.concatenate([-sinT[0:HD // 2], sinT[HD // 2:]], axis=0)
    sgnT = np.ascontiguousarray(sgnT)

    m = np.asarray(attention_mask, np.float32)[0, 0]
    expected = np.where(np.tril(np.ones((S, S), bool)), 0.0, NEG).astype(np.float32)
    causal = bool(np.array_equal(m, expected))

    TC = T // n_cores
    QB = min(512, TC)
    KB = QB // 128
    qkd = (lambda x: _np8(x * S_W)) if fp8 else (lambda x: _np16(x))
    in_maps = []
    for c in range(n_cores):
        im = {
            "hsTt": hsTt,
            "wqTt": _tile_w_hm(qkd(np.asarray(wq, np.float32)[c * D:(c + 1) * D].T),
                               HT, D),
            "wkTt": _tile_w(qkd(np.asarray(wk, np.float32)[c * KD:(c + 1) * KD].T),
                            HT, HD),
            "wvTt": _tile_w(_np16(np.asarray(wv, np.float32)[c * KD:(c + 1) * KD].T),
                            HT, HD),
            "woTt": woTt,
            "cosT": cosT,
            "sgnT": sgnT,
            "idn": _np16(np.eye(128, dtype=np.float32)),
        }
        if fp8:
            im["hsT8t"] = _tile_hs(_np8(hs2.T * S_HS), NIT, TI, nh2)
        if causal:
            pk = np.arange(128)[:, None]
            pq = np.arange(QB)[None, :]
            dmask = np.concatenate(
                [np.where(pk + j * 128 <= pq, 1.0, 0.0) for j in range(KB)],
                axis=1).astype(np.float32)
            im["dmask"] = _np16(dmask)
        else:
            im["maskT"] = np.ascontiguousarray(m.T)
        in_maps.append(im)
    return in_maps, causal, (T, S, H)


_BUILD_CACHE = {}


def kernel(hidden_states, attention_mask, cos, sin, wq, wk, wv, wo,
           compute_dtype="fp8", trace=False):
    B, S, H = hidden_states.shape
    T = B * S
    in_maps, causal, dims = prep_inputs(
        hidden_states, attention_mask, cos, sin, wq, wk, wv, wo,
        compute_dtype=compute_dtype)
    key = (T, S, H, compute_dtype, causal)
    if key not in _BUILD_CACHE:
        _BUILD_CACHE[key] = build(T, S, H, compute_dtype=compute_dtype,
                                  causal=causal)
    nc = _BUILD_CACHE[key]
    res = run_bass_kernel_spmd(nc, in_maps, core_ids=list(range(N_CORES)),
                               trace=trace)
    TC = T // N_CORES
    TB = TC // B
    out = np.empty((T, H), np.float32)
    for c in range(N_CORES):
        oc = res.results[c]["out"]
        for b in range(B):
            out[b * S + c * TB: b * S + (c + 1) * TB] = \
                oc[b * TB:(b + 1) * TB]
    if trace:
        kernel.last_exec_time_ns = res.exec_time_ns
        kernel.last_results = res
    return out.reshape(B, S, H)
